# revision 1
# baseline (speedup 1.0000x reference)
"""CIN (Compressed Interaction Network) kernel for Trainium2, 8-core data parallel.

Reference computation (per batch element b, position d):
  hidden = x                                  # (39 fields)
  layer i: z[(m,n)] = x[m] * hidden[n]        # outer product over fields
           cur[o]   = relu(sum_c z[c] W_i[c,o] + b_i[o])   # 200 outs
           hidden, direct = cur[:100], cur[100:]  (layers 0,1);  direct = cur (layer 2)
  out[b, j] = sum_d concat(directs)[j, d]     # (2048, 400)

Strategy: batch sharded across 8 cores (256 batch each, rows = b*32+d -> 8192).
Channel-major layout everywhere: X (39p, rows), H (100p, rows), all bf16.
z built on VectorE as XR (X broadcast across partitions via DMA from DRAM)
times H (free-dim broadcast).  TensorE contracts z blocks against the
(statically reordered) weights with PSUM accumulation; ScalarE applies
bias+ReLU+cast; VectorE reduces over d into the output accumulator.
"""

import sys

sys.path.insert(0, '/opt/trn_rl_repo')

import numpy as np
import ml_dtypes

import concourse.bacc as bacc
import concourse.mybir as mybir
import concourse.tile as tile
from concourse import bass_utils

BF16 = ml_dtypes.bfloat16

NCORES = 8
B = 2048
BC = B // NCORES          # 256 batch per core
D = 32
ROWS = BC * D             # 8192
F0 = 39
FK = 100
O = 200
RT = 512                  # rows per tile
NRT = ROWS // RT          # 16
BPT = RT // D             # batches per row tile = 16
G0 = 3                    # layer-0: 3 m-groups per K-block
P0 = G0 * F0              # 117
KB0 = F0 // G0            # 13
BL = 125                  # layers-1/2: packed K-block size (4-phase H windows)
NBL = 32                  # ceil(3900 / BL)
PAD_CH = BL * NBL         # 4000 (rows 3900.. are zero-padded in W)
NPH = 4                   # H phase offsets {0, 25, 50, 75}
BPP = NBL // NPH          # blocks per phase = 8
PERM = [q + NPH * i for q in range(NPH) for i in range(BPP)]  # phase-major block order
KZ0 = 4                   # layer-0 z first-chunk blocks (early PE start)

_cached = {}


def _emit(tc, outs, ins):
    nc = tc.nc
    x_d = ins['x_t']
    x0_d = ins['x0']
    xp_d = ins['xrp']
    w0_d = ins['w0']
    w1_d = ins['w1']
    w2_d = ins['w2']
    b_d = ins['bias']
    sel_d = ins['sel']
    out_d = outs['out']

    bf = mybir.dt.bfloat16
    f32 = mybir.dt.float32
    mult = mybir.AluOpType.mult
    add = mybir.AluOpType.add
    relu = mybir.ActivationFunctionType.Relu
    X = mybir.AxisListType.X

    import contextlib
    ctx = contextlib.ExitStack()
    with ctx:
        const = ctx.enter_context(tc.tile_pool(name="const", bufs=1))
        accp = ctx.enter_context(tc.tile_pool(name="acc", bufs=1))
        xrp = ctx.enter_context(tc.tile_pool(name="xr", bufs=2))
        xr0p = ctx.enter_context(tc.tile_pool(name="xr0", bufs=1))
        xt0p = ctx.enter_context(tc.tile_pool(name="xt0", bufs=1))
        z0p = ctx.enter_context(tc.tile_pool(name="z0", bufs=1))
        zp = ctx.enter_context(tc.tile_pool(name="z", bufs=5))
        hp = ctx.enter_context(tc.tile_pool(name="h", bufs=3))
        dp = ctx.enter_context(tc.tile_pool(name="d", bufs=2))
        psum = ctx.enter_context(tc.tile_pool(name="ps", bufs=6, space="PSUM"))
        selps = ctx.enter_context(tc.tile_pool(name="selps", bufs=2, space="PSUM"))

        # resident weights / bias (DMAs emitted after the critical rt=0 loads)
        w0_sb = const.tile([P0, KB0, O], bf, tag="w0")
        w1_sb = const.tile([BL, NBL, O], bf, tag="w1")
        w2_sb = const.tile([BL, NBL, O], bf, tag="w2")
        b_sb = const.tile([FK, 6], f32, tag="bias")
        sel_sb = const.tile([FK, NPH, BL], bf, tag="sel")

        # output accumulators (j-group on partitions, batch on free)
        acc = [accp.tile([FK, BC], f32, tag=f"acc{i}", name=f"acc{i}") for i in range(4)]

        def load_l0(rt, after_first=None):
            xr0 = xr0p.tile([P0, KB0, RT], bf, tag="xr0", name=f"xr0_{rt}")
            for dmi in range(G0):
                nc.sync.dma_start(
                    xr0[dmi * F0:(dmi + 1) * F0, :KZ0, :],
                    x0_d[rt, dmi, :KZ0][None, :, :].to_broadcast((F0, KZ0, RT)))
            if after_first is not None:
                after_first()
            for dmi in range(G0):
                nc.sync.dma_start(
                    xr0[dmi * F0:(dmi + 1) * F0, KZ0:, :],
                    x0_d[rt, dmi, KZ0:][None, :, :].to_broadcast((F0, KB0 - KZ0, RT)))
            xt0 = xt0p.tile([P0, RT], bf, tag="xt0", name=f"xt0_{rt}")
            nc.scalar.dma_start(xt0, x_d[rt][None, :, :].to_broadcast((G0, F0, RT)))
            return xr0, xt0

        def build_z0(l0t, rt):
            xr0, xt0 = l0t
            z0 = z0p.tile([P0, KB0, RT], bf, tag="z0", name=f"z0_{rt}")
            nc.vector.tensor_tensor(
                z0[:, :KZ0, :], xr0[:, :KZ0, :],
                xt0[:, None, :].to_broadcast((P0, KZ0, RT)), mult)
            nc.vector.tensor_tensor(
                z0[:, KZ0:, :], xr0[:, KZ0:, :],
                xt0[:, None, :].to_broadcast((P0, KB0 - KZ0, RT)), mult)
            return z0

        def load_xr(rt):
            xr = xrp.tile([BL, NBL, RT], bf, tag="xr", name=f"xr_{rt}")
            for q in range(NPH):
                sl = slice(q * BPP, (q + 1) * BPP)
                nc.gpsimd.dma_start(xr[:, sl, :], xp_d[rt, :, sl, :])
            return xr

        def make_phases(h_sb, li, rt):
            """ph_q[p] = H[(25q + p) mod 100] via 0/1 selection matmuls (exact)."""
            phs = []
            for q in range(NPH):
                psq = selps.tile([BL, RT], f32, tag="selps", name=f"sps{q}_{li}_{rt}")
                nc.tensor.matmul(psq, sel_sb[:, q, :], h_sb, start=True, stop=True)
                t = hp.tile([BL, RT], bf, tag=f"ph{q}", name=f"ph{q}_{li}_{rt}")
                nc.scalar.copy(t, psq)
                phs.append(t)
            return phs

        def l0_section(z0, rt):
            """L0 matmuls + relu + d0-reduce for row-tile rt; returns h1."""
            bs = slice(rt * BPT, (rt + 1) * BPT)
            ps0 = [psum.tile([FK, RT], f32, tag="ps", name=f"ps0_{rt}_{t}")
                   for t in range(2)]
            for t in range(2):
                for kb in range(KB0):
                    nc.tensor.matmul(ps0[t], w0_sb[:, kb, t * FK:(t + 1) * FK],
                                     z0[:, kb, :], start=(kb == 0),
                                     stop=(kb == KB0 - 1))
            h1 = hp.tile([FK, RT], bf, tag="h", name=f"h1_{rt}")
            nc.scalar.activation(h1, ps0[0], relu, bias=b_sb[:, 0:1])
            h1phs = make_phases(h1, 0, rt)
            d0 = dp.tile([FK, RT], bf, tag="d", name=f"d0_{rt}")
            nc.scalar.activation(d0, ps0[1], relu, bias=b_sb[:, 1:2])

            def d0_reduce():
                nc.vector.tensor_reduce(
                    acc[0][:, bs], d0.rearrange("o (g f) -> o g f", f=D), X, add)
            return h1phs, d0_reduce

        def layer_section(li, rt, xr, hphs, mid_hook=None):
            """Packed z-blocks + matmuls for layer li+1 (1 or 2) of row-tile rt."""
            bs = slice(rt * BPT, (rt + 1) * BPT)
            w_sb = w1_sb if li == 0 else w2_sb
            bcol = 2 + 2 * li
            ps = [psum.tile([FK, RT], f32, tag="ps", name=f"psl{li}_{rt}_{t}")
                  for t in range(2)]
            for q in range(NPH):
                zt = zp.tile([BL, BPP, RT], bf, tag="z", name=f"z{li}_{rt}_{q}")
                hb = hphs[q]
                xq = xr[:, q * BPP:(q + 1) * BPP, :]
                if q == 0:
                    # small first chunk so the PE refills quickly after the barrier
                    nc.vector.tensor_tensor(
                        zt[:, :2, :], xq[:, :2, :],
                        hb[:, None, :].to_broadcast((BL, 2, RT)), mult)
                    nc.vector.tensor_tensor(
                        zt[:, 2:, :], xq[:, 2:, :],
                        hb[:, None, :].to_broadcast((BL, BPP - 2, RT)), mult)
                else:
                    nc.vector.tensor_tensor(
                        zt, xq, hb[:, None, :].to_broadcast((BL, BPP, RT)), mult)
                for t in range(2):
                    for i in range(BPP):
                        kq = q * BPP + i
                        nc.tensor.matmul(ps[t], w_sb[:, kq, t * FK:(t + 1) * FK],
                                         zt[:, i, :], start=(q == 0 and i == 0),
                                         stop=(q == NPH - 1 and i == BPP - 1))
                if q == 1 and mid_hook is not None:
                    mid_hook()
            if li == 0:
                h2 = hp.tile([FK, RT], bf, tag="h", name=f"h2_{rt}")
                nc.scalar.activation(h2, ps[0], relu, bias=b_sb[:, bcol:bcol + 1])
                d1 = dp.tile([FK, RT], bf, tag="d", name=f"d1_{rt}")
                nc.scalar.activation(d1, ps[1], relu, bias=b_sb[:, bcol + 1:bcol + 2])
                nc.vector.tensor_reduce(
                    acc[1][:, bs], d1.rearrange("o (g f) -> o g f", f=D), X, add)
                return lambda: make_phases(h2, 1, rt)
            for t in range(2):
                d2 = dp.tile([FK, RT], bf, tag="d", name=f"d2_{rt}_{t}")
                nc.scalar.activation(d2, ps[t], relu,
                                     bias=b_sb[:, bcol + t:bcol + t + 1])
                nc.vector.tensor_reduce(
                    acc[2 + t][:, bs], d2.rearrange("o (g f) -> o g f", f=D),
                    X, add)
            return None

        # ---- software pipeline: [L1(rt) | L0(rt+1) | L2(rt)] per iteration
        l0t = load_l0(0, after_first=lambda: nc.sync.dma_start(w0_sb, w0_d))
        nc.sync.dma_start(b_sb, b_d)
        nc.sync.dma_start(sel_sb, sel_d)
        nc.scalar.dma_start(w1_sb, w1_d)
        nc.scalar.dma_start(w2_sb, w2_d)
        xr_cur = load_xr(0)
        z0_cur = build_z0(l0t, 0)
        h1, d0red = l0_section(z0_cur, 0)
        d0red()
        l0t_next = load_l0(1)
        state = {'z0_next': None}
        d0red_next = None

        for rt in range(NRT):
            xr_next = load_xr(rt + 1) if rt + 1 < NRT else None

            def mid_hook(rt=rt):
                if rt + 1 < NRT:
                    state['z0_next'] = build_z0(l0t_next, rt + 1)

            h2phs_fn = layer_section(0, rt, xr_cur, h1, mid_hook=mid_hook)
            h2phs = h2phs_fn()
            if rt + 1 < NRT:
                h1, d0red_next = l0_section(state['z0_next'], rt + 1)
                if rt + 2 < NRT:
                    l0t_next = load_l0(rt + 2)
            else:
                d0red_next = None
            layer_section(1, rt, xr_cur, h2phs, mid_hook=d0red_next)
            xr_cur = xr_next

        for i in range(4):
            nc.sync.dma_start(out_d[i * FK:(i + 1) * FK, :], acc[i])


def _pack_w(W):
    Wp = np.zeros((PAD_CH, O), np.float32)
    Wp[:F0 * FK] = W
    return np.ascontiguousarray(
        Wp.reshape(NBL, BL, O)[PERM].transpose(1, 0, 2)).astype(BF16)


def _prep_weights(W0, W1, W2, b0, b1, b2):
    w0 = np.ascontiguousarray(
        W0.reshape(KB0, P0, O).transpose(1, 0, 2)).astype(BF16)
    w1 = _pack_w(W1)
    w2 = _pack_w(W2)
    bias = np.ascontiguousarray(
        np.stack([b0, b1, b2]).reshape(3, 2, FK).transpose(2, 0, 1).reshape(FK, 6)
    ).astype(np.float32)
    sel = np.zeros((FK, NPH, BL), np.float32)
    q_, p_ = np.meshgrid(np.arange(NPH), np.arange(BL), indexing='ij')
    sel[(25 * q_ + p_) % FK, q_, p_] = 1.0
    return w0, w1, w2, bias, sel.astype(BF16)


def _prep_x_shard(x, c):
    xs = x[c * BC:(c + 1) * BC]                      # (BC, 39, 32)
    xt = xs.transpose(1, 0, 2).reshape(F0, ROWS)          # (39, 8192)
    x_t = np.ascontiguousarray(
        xt.reshape(F0, NRT, RT).transpose(1, 0, 2)).astype(BF16)
    x_perm = np.ascontiguousarray(
        xt.reshape(KB0, G0, NRT, RT).transpose(2, 1, 0, 3)).astype(BF16)
    mmap = np.minimum(np.arange(PAD_CH) // FK, F0 - 1).reshape(NBL, BL)[PERM]
    xrp = np.ascontiguousarray(
        x_t[:, mmap, :].transpose(0, 2, 1, 3))     # (NRT, BL, NBL, RT) phase-major
    return {'x_t': x_t, 'x0': x_perm, 'xrp': xrp}


def _build():
    if 'nc' in _cached:
        return _cached['nc']
    nc = bacc.Bacc("TRN2", target_bir_lowering=False, debug=False,
                   enable_asserts=False, num_devices=NCORES)
    ins = {
        'x_t': nc.dram_tensor("x_t", (NRT, F0, RT), mybir.dt.bfloat16,
                              kind="ExternalInput").ap(),
        'x0': nc.dram_tensor("x0", (NRT, G0, KB0, RT), mybir.dt.bfloat16,
                             kind="ExternalInput").ap(),
        'xrp': nc.dram_tensor("xrp", (NRT, BL, NBL, RT), mybir.dt.bfloat16,
                              kind="ExternalInput").ap(),
        'w0': nc.dram_tensor("w0", (P0, KB0, O), mybir.dt.bfloat16,
                             kind="ExternalInput").ap(),
        'w1': nc.dram_tensor("w1", (BL, NBL, O), mybir.dt.bfloat16,
                             kind="ExternalInput").ap(),
        'w2': nc.dram_tensor("w2", (BL, NBL, O), mybir.dt.bfloat16,
                             kind="ExternalInput").ap(),
        'bias': nc.dram_tensor("bias", (FK, 6), mybir.dt.float32,
                               kind="ExternalInput").ap(),
        'sel': nc.dram_tensor("sel", (FK, NPH, BL), mybir.dt.bfloat16,
                              kind="ExternalInput").ap(),
    }
    outs = {
        'out': nc.dram_tensor("out", (4 * FK, BC), mybir.dt.float32,
                              kind="ExternalOutput").ap(),
    }
    with tile.TileContext(nc, trace_sim=False) as tc:
        _emit(tc, outs, ins)
    nc.compile()
    _cached['nc'] = nc
    return nc


def kernel(x, W0, W1, W2, b0, b1, b2):
    nc = _build()
    w0, w1, w2, bias, sel = _prep_weights(
        np.asarray(W0, np.float32), np.asarray(W1, np.float32),
        np.asarray(W2, np.float32), np.asarray(b0, np.float32),
        np.asarray(b1, np.float32), np.asarray(b2, np.float32))
    x = np.asarray(x, np.float32)
    in_maps = []
    for c in range(NCORES):
        in_maps.append({
            **_prep_x_shard(x, c),
            'w0': w0, 'w1': w1, 'w2': w2, 'bias': bias, 'sel': sel,
        })
    res = bass_utils.run_bass_kernel_spmd(
        nc, in_maps, core_ids=list(range(NCORES)))
    out = np.empty((B, 4 * FK), np.float32)
    for c in range(NCORES):
        out[c * BC:(c + 1) * BC, :] = res.results[c]['out'].T
    return out

